# revision 1
# baseline (speedup 1.0000x reference)
"""ChebConv(K=5) + Linear + log_softmax GNN kernel for 8 Trainium2 NeuronCores.

Strategy (graph/data parallel, per sharding hint):
 - Nodes are sharded by destination across 8 cores (6250 nodes each, padded
   to S=6272 rows). Within each core, nodes are permuted so rows are grouped
   by in-degree class; all index structure is precomputed on the host.
 - The normalized propagation  prop(h) = -D^-1/2 A D^-1/2 h  is refactored
   so the device only ever computes raw gather-sums  s[dst] = sum t[src]:
   the table stores t_k = D^-1/2 T_k(L~)x, and the Chebyshev recurrence
   becomes  t_{k+1} = -2 D^-1 s - t_{k-1}  (first hop: t_1 = -D^-1 s).
 - Each hop: every core gathers its edges' source rows from a replicated
   bf16 table in HBM via gpsimd dma_gather (one call per destination tile
   per index range), reduces them per destination with TensorE matmuls
   against precomputed 0/1 "segment" patterns accumulating in PSUM, applies
   the recurrence on VectorE, and contributes its updated slice to the next
   table via an 8-core AllGather (runs on TOPSP/SDMA, overlapped).
 - Because dma_gather indices are int16, the table is split into a "lo"
   region (cores 0-4) addressed from row 0 and a "hi" region (cores 5-7)
   addressed from row 31361; each node's edges are split accordingly.
 - The output projection z = sum_k (sqrt(D) t_k) W_k is accumulated per hop
   (PE transpose + fp32 matmul), followed by relu, the 50->10 Linear and a
   row-wise log_softmax.
"""
import numpy as np
import ml_dtypes

bf16 = ml_dtypes.bfloat16

# ---------------- problem constants (hardcoded per contract) ---------------
N = 50000
E = 1_600_000
D = 128
K = 5
NCORES = 8
NPC = N // NCORES            # 6250
TILES = (NPC + 127) // 128   # 49
S = TILES * 128              # 6272
LO_CORES = 5
LO_SPLIT_NODE = LO_CORES * NPC       # 31250
HI_BASE_ROW = 1 + LO_CORES * S       # 31361
TOT_ROWS = 2 + NCORES * S            # 50178
HI_PAD_IDX = TOT_ROWS - 1 - HI_BASE_ROW  # 18816 -> trailing zero row
ALLOWED_C = np.array([8, 16, 24, 32, 40, 48, 56, 64, 80, 96, 128], dtype=np.int64)
BASE_PID = [1 + S * c for c in range(LO_CORES)] + [
    HI_BASE_ROW + S * (c - LO_CORES) for c in range(LO_CORES, NCORES)
]


def _class_of(d):
    """Smallest allowed class >= d (d: int64 array)."""
    idx = np.searchsorted(ALLOWED_C, d)
    out = ALLOWED_C[np.minimum(idx, len(ALLOWED_C) - 1)]
    assert (out >= d).all(), "degree exceeds max class"
    return np.where(d == 0, 0, out)


def host_prep(x, edge_index):
    row = np.ascontiguousarray(edge_index[0]).astype(np.int64)
    col = np.ascontiguousarray(edge_index[1]).astype(np.int64)
    deg = np.bincount(row, minlength=N)
    assert (deg > 0).all(), "kernel assumes no isolated (deg-0) nodes"
    degf = deg.astype(np.float32)
    dinv = (1.0 / np.sqrt(degf)).astype(np.float32)

    is_lo = col < LO_SPLIT_NODE
    lodeg = np.bincount(row[is_lo], minlength=N)
    hideg = deg - lodeg
    cl = _class_of(lodeg)
    ch = _class_of(hideg)

    # per-core permutation: sort nodes by (cl desc, ch desc)
    perms = np.empty((NCORES, NPC), dtype=np.int64)
    for c in range(NCORES):
        ids = np.arange(c * NPC, (c + 1) * NPC)
        order = np.lexsort((-ch[ids], -cl[ids]))
        perms[c] = ids[order]

    rank = np.empty(N, dtype=np.int64)
    rank[perms.reshape(-1)] = np.tile(np.arange(NPC), NCORES)
    pid = np.asarray(BASE_PID, dtype=np.int64)[np.arange(N) // NPC] + rank

    # common tiling: per tile, max class over all cores (padded rows class 0)
    clp = np.zeros((NCORES, S), dtype=np.int64)
    chp = np.zeros((NCORES, S), dtype=np.int64)
    for c in range(NCORES):
        clp[c, :NPC] = cl[perms[c]]
        chp[c, :NPC] = ch[perms[c]]
    CLO = clp.reshape(NCORES, TILES, 128).max(axis=(0, 2))
    CHI = chp.reshape(NCORES, TILES, 128).max(axis=(0, 2))
    lo_tile_off = np.zeros(TILES + 1, dtype=np.int64)
    hi_tile_off = np.zeros(TILES + 1, dtype=np.int64)
    np.cumsum(CLO * 128, out=lo_tile_off[1:])
    np.cumsum(CHI * 128, out=hi_tile_off[1:])
    n_lo, n_hi = int(lo_tile_off[-1]), int(hi_tile_off[-1])

    # per-node slot bases (in its core's slot array)
    tile_of_rank = np.arange(NPC) // 128
    row_in_tile = np.arange(NPC) % 128
    lo_base_rank = lo_tile_off[tile_of_rank] + row_in_tile * CLO[tile_of_rank]
    hi_base_rank = hi_tile_off[tile_of_rank] + row_in_tile * CHI[tile_of_rank]
    lo_base = np.empty(N, dtype=np.int64)
    hi_base = np.empty(N, dtype=np.int64)
    lo_base[perms.reshape(-1)] = np.tile(lo_base_rank, NCORES)
    hi_base[perms.reshape(-1)] = np.tile(hi_base_rank, NCORES)

    # edges sorted by dst; per-edge rank among same-(dst,pass) edges
    order_e = np.argsort(row, kind="stable")
    row_s, col_s = row[order_e], col[order_e]
    is_lo_s = is_lo[order_e]
    # occurrence index within dst for lo and hi subsets separately
    estart = np.zeros(N + 1, dtype=np.int64)
    np.cumsum(deg, out=estart[1:])

    def occ_index(dst_sub, count_sub):
        st = np.zeros(N + 1, dtype=np.int64)
        np.cumsum(count_sub, out=st[1:])
        return np.arange(dst_sub.shape[0], dtype=np.int64) - st[dst_sub]

    dst_lo, src_lo = row_s[is_lo_s], col_s[is_lo_s]
    dst_hi, src_hi = row_s[~is_lo_s], col_s[~is_lo_s]
    j_lo = occ_index(dst_lo, np.bincount(dst_lo, minlength=N))
    j_hi = occ_index(dst_hi, np.bincount(dst_hi, minlength=N))

    idx_lo = np.zeros((NCORES, n_lo), dtype=np.int16)             # pad -> row 0
    idx_hi = np.full((NCORES, n_hi), HI_PAD_IDX, dtype=np.int16)  # pad -> zero row
    core_lo, core_hi = dst_lo // NPC, dst_hi // NPC
    slot_lo = lo_base[dst_lo] + j_lo
    slot_hi = hi_base[dst_hi] + j_hi
    v_lo = pid[src_lo]
    v_hi = pid[src_hi] - HI_BASE_ROW
    assert v_lo.max() <= 32767 and v_lo.min() >= 1
    assert v_hi.max() <= 32767 and v_hi.min() >= 0
    idx_lo[core_lo, slot_lo] = v_lo.astype(np.int16)
    idx_hi[core_hi, slot_hi] = v_hi.astype(np.int16)

    # wrap to dma_gather layout [128, n/16] (16-partition stripes, 8 replicas)
    def wrap(a):
        t = a.reshape(-1, 16).T          # [16, n/16]
        return np.ascontiguousarray(np.tile(t, (8, 1)))

    idx_lo_w = np.stack([wrap(idx_lo[c]) for c in range(NCORES)])
    idx_hi_w = np.stack([wrap(idx_hi[c]) for c in range(NCORES)])

    # patterns: full-height [128, 128] bf16 one-hot per (class, chunk j)
    pat_pool, pat_list = {}, []
    chunk_meta = {}
    for cval in sorted(set(CLO.tolist()) | set(CHI.tolist())):
        if cval == 0:
            continue
        metas = []
        e = np.arange(128)
        for j in range(int(cval)):
            d = (128 * j + e) // cval
            assert (d < 128).all()
            P = np.zeros((128, 128), dtype=bf16)
            P[e, d] = 1
            key = (int(cval), int(j))
            pat_pool[key] = len(pat_list)
            pat_list.append(P)
            metas.append(pat_pool[key])
        chunk_meta[int(cval)] = metas
    pats = np.stack(pat_list)  # [NPAT, 128, 128]

    # per-row constants in [128, TILES] layout (value for row g at [g%128, g//128])
    def rowconst(vals_percore):  # [NCORES, S] f32 -> [NCORES, 128, TILES]
        return np.ascontiguousarray(
            vals_percore.reshape(NCORES, TILES, 128).transpose(0, 2, 1))

    dinv_p = np.zeros((NCORES, S), dtype=np.float32)
    sdeg_p = np.zeros((NCORES, S), dtype=np.float32)
    for c in range(NCORES):
        dinv_p[c, :NPC] = dinv[perms[c]]
        sdeg_p[c, :NPC] = np.sqrt(degf[perms[c]])
    di2 = dinv_p * dinv_p

    xp = np.zeros((NCORES, S, D), dtype=np.float32)
    for c in range(NCORES):
        xp[c, :NPC] = x[perms[c]]

    return dict(
        perms=perms, CLO=CLO, CHI=CHI,
        lo_tile_off=lo_tile_off, hi_tile_off=hi_tile_off,
        n_lo=n_lo, n_hi=n_hi,
        idx_lo_w=idx_lo_w, idx_hi_w=idx_hi_w,
        pats=pats, chunk_meta=chunk_meta,
        xp=xp,
        dinv_t=rowconst(dinv_p),
        m1di2_t=rowconst(-di2),
        m2di2_t=rowconst(-2.0 * di2),
        sdeg_t=rowconst(sdeg_p),
    )


def build_nc(meta, cheb_w, cheb_b, fc_w, fc_b):
    from concourse import bacc, mybir
    import concourse.tile as tile

    f32, bft, i16 = mybir.dt.float32, mybir.dt.bfloat16, mybir.dt.int16
    CLO, CHI = meta["CLO"], meta["CHI"]
    cm = meta["chunk_meta"]
    n_lo, n_hi = meta["n_lo"], meta["n_hi"]
    NPAT = meta["pats"].shape[0]
    CLO_MAX, CHI_MAX = int(CLO.max()), int(CHI.max())

    nc = bacc.Bacc(target_bir_lowering=False, num_swdge_queues=2)

    # ---- I/O --------------------------------------------------------------
    xp_d = nc.declare_dram_parameter("xp", [S, D], f32, isOutput=False)
    il_d = nc.declare_dram_parameter("idx_lo", [128, n_lo // 16], i16, isOutput=False)
    ih_d = nc.declare_dram_parameter("idx_hi", [128, n_hi // 16], i16, isOutput=False)
    pat_d = nc.declare_dram_parameter("pats", [NPAT * 128, 128], bft, isOutput=False)
    dinv_d = nc.declare_dram_parameter("dinv_t", [128, TILES], f32, isOutput=False)
    m1_d = nc.declare_dram_parameter("m1di2_t", [128, TILES], f32, isOutput=False)
    m2_d = nc.declare_dram_parameter("m2di2_t", [128, TILES], f32, isOutput=False)
    sdeg_d = nc.declare_dram_parameter("sdeg_t", [128, TILES], f32, isOutput=False)
    wch_d = nc.declare_dram_parameter("wcheb", [128, K * 50], f32, isOutput=False)
    cb_d = nc.declare_dram_parameter("cbias", [50, 1], f32, isOutput=False)
    fw_d = nc.declare_dram_parameter("fcw", [50, 10], f32, isOutput=False)
    fb_d = nc.declare_dram_parameter("fcb_rep", [128, 10], f32, isOutput=False)
    id_d = nc.declare_dram_parameter("ident", [128, 128], f32, isOutput=False)
    out_d = nc.declare_dram_parameter("out", [S, 10], f32, isOutput=True)

    # ---- internal DRAM ----------------------------------------------------
    agin = [nc.dram_tensor(f"agin{k}", [S, D], bft) for k in range(K - 1)]
    tables = [
        nc.dram_tensor(f"table{k}", [TOT_ROWS, D], bft, addr_space="Shared")
        for k in range(K - 1)
    ]

    with tile.TileContext(nc) as tc:
        with tc.tile_pool(name="cst", bufs=1) as cst, \
             tc.tile_pool(name="xt", bufs=3) as xtp, \
             tc.tile_pool(name="glo", bufs=2) as glop, \
             tc.tile_pool(name="ghi", bufs=2) as ghip, \
             tc.tile_pool(name="st", bufs=3) as stp, \
             tc.tile_pool(name="fin", bufs=2) as finp, \
             tc.tile_pool(name="ps_s", bufs=2, space="PSUM") as ps_s, \
             tc.tile_pool(name="ps_t", bufs=2, space="PSUM") as ps_t, \
             tc.tile_pool(name="ps_z", bufs=2, space="PSUM") as ps_z:

            # ---- resident constants --------------------------------------
            idx_lo_s = cst.tile([128, n_lo // 16], i16)
            idx_hi_s = cst.tile([128, n_hi // 16], i16)
            nc.sync.dma_start(out=idx_lo_s[:], in_=il_d[:, :])
            nc.sync.dma_start(out=idx_hi_s[:], in_=ih_d[:, :])
            pats_s = cst.tile([128, NPAT, 128], bft)
            nc.sync.dma_start(
                out=pats_s[:],
                in_=pat_d[:, :].rearrange("(n p) d -> p n d", p=128),
            )
            ident = cst.tile([128, 128], f32)
            nc.sync.dma_start(out=ident[:], in_=id_d[:, :])
            dinv_s = cst.tile([128, TILES], f32)
            nc.sync.dma_start(out=dinv_s[:], in_=dinv_d[:, :])
            m1_s = cst.tile([128, TILES], f32)
            nc.sync.dma_start(out=m1_s[:], in_=m1_d[:, :])
            m2_s = cst.tile([128, TILES], f32)
            nc.sync.dma_start(out=m2_s[:], in_=m2_d[:, :])
            sdeg_s = cst.tile([128, TILES], f32)
            nc.sync.dma_start(out=sdeg_s[:], in_=sdeg_d[:, :])
            wch_s = cst.tile([128, K * 50], f32)
            nc.sync.dma_start(out=wch_s[:], in_=wch_d[:, :])
            cb_s = cst.tile([50, 1], f32)
            nc.sync.dma_start(out=cb_s[:], in_=cb_d[:, :])
            fw_s = cst.tile([50, 10], f32)
            nc.sync.dma_start(out=fw_s[:], in_=fw_d[:, :])
            fb_s = cst.tile([128, 10], f32)
            nc.sync.dma_start(out=fb_s[:], in_=fb_d[:, :])

            gA = cst.tile([128, TILES, 128], f32)   # holds t_{k-1} slices
            gB = cst.tile([128, TILES, 128], f32)   # holds t_k slices
            z_s = cst.tile([50, S], f32)            # z^T accumulator

            # zero rows of each table
            zrow = cst.tile([1, D], bft)
            nc.vector.memset(zrow[:], 0.0)
            for t in tables:
                nc.sync.dma_start(out=t[0:1, :], in_=zrow[:])
                nc.sync.dma_start(out=t[TOT_ROWS - 1 : TOT_ROWS, :], in_=zrow[:])

            def z_project(k, src_tile, t):
                """z[:, tile t] (+)= W_k^T @ (sdeg * src_tile)^T"""
                zsc = stp.tile([128, 128], f32, tag="zsc")
                nc.vector.tensor_scalar_mul(
                    out=zsc[:], in0=src_tile, scalar1=sdeg_s[:, t : t + 1])
                tp = ps_t.tile([128, 128], f32, space="PSUM")
                nc.tensor.transpose(out=tp[:], in_=zsc[:], identity=ident[:])
                trs = stp.tile([128, 128], f32, tag="trs")
                nc.vector.tensor_copy(out=trs[:], in_=tp[:])
                zp = ps_z.tile([50, 128], f32, space="PSUM")
                nc.tensor.matmul(out=zp[:], lhsT=wch_s[:, 50 * k : 50 * (k + 1)],
                                 rhs=trs[:], start=True, stop=True)
                zsl = z_s[:, 128 * t : 128 * (t + 1)]
                if k == 0:
                    nc.vector.tensor_copy(out=zsl, in_=zp[:])
                else:
                    nc.vector.tensor_tensor(
                        out=zsl, in0=zsl, in1=zp[:], op=mybir.AluOpType.add)

            # ---- prologue: t_0 = dinv * x, table0, z += W_0 term ---------
            for t in range(TILES):
                xt = xtp.tile([128, D], f32)
                nc.sync.dma_start(out=xt[:], in_=xp_d[128 * t : 128 * (t + 1), :])
                ga = gA[:, t, :]
                nc.vector.tensor_scalar_mul(
                    out=ga, in0=xt[:], scalar1=dinv_s[:, t : t + 1])
                xb = stp.tile([128, D], bft, tag="xb")
                nc.scalar.activation(out=xb[:], in_=ga,
                                     func=mybir.ActivationFunctionType.Copy)
                nc.sync.dma_start(out=agin[0][128 * t : 128 * (t + 1), :], in_=xb[:])
                z_project(0, ga, t)
            nc.gpsimd.collective_compute(
                "AllGather", mybir.AluOpType.bypass,
                replica_groups=[list(range(NCORES))],
                ins=[agin[0][:, :]], outs=[tables[0][1 : TOT_ROWS - 1, :]],
            )

            # ---- hops ----------------------------------------------------
            for k in range(1, K):
                tbl = tables[k - 1]
                tbl_hi = tbl[HI_BASE_ROW:TOT_ROWS, :]
                for t in range(TILES):
                    clo, chi = int(CLO[t]), int(CHI[t])
                    chunks = []  # (pattern_id, G_view)
                    if clo:
                        gl = glop.tile([128, CLO_MAX, 128], bft)
                        o16 = int(meta["lo_tile_off"][t]) // 16
                        nc.gpsimd.dma_gather(
                            out_ap=gl[:, :clo, :],
                            in_ap=tbl[:, :],
                            idxs_ap=idx_lo_s[:, o16 : o16 + clo * 8],
                            num_idxs=clo * 128, num_idxs_reg=clo * 128,
                            elem_size=D, queue_num=0, single_packet=False,
                        )
                        chunks += [(cm[clo][j], gl[:, j, :]) for j in range(clo)]
                    if chi:
                        gh = ghip.tile([128, CHI_MAX, 128], bft)
                        o16 = int(meta["hi_tile_off"][t]) // 16
                        nc.gpsimd.dma_gather(
                            out_ap=gh[:, :chi, :],
                            in_ap=tbl_hi,
                            idxs_ap=idx_hi_s[:, o16 : o16 + chi * 8],
                            num_idxs=chi * 128, num_idxs_reg=chi * 128,
                            elem_size=D, queue_num=1, single_packet=False,
                        )
                        chunks += [(cm[chi][j], gh[:, j, :]) for j in range(chi)]

                    sp = ps_s.tile([128, 128], f32, space="PSUM")
                    nch = len(chunks)
                    for i, (pid_, gv) in enumerate(chunks):
                        nc.tensor.matmul(
                            out=sp[:], lhsT=pats_s[:, pid_, :], rhs=gv,
                            start=(i == 0), stop=(i == nch - 1),
                            skip_group_check=True,
                        )

                    # recurrence
                    dst = gB[:, t, :] if k % 2 == 1 else gA[:, t, :]
                    prv = dst  # t_{k-2} lives in the buffer being overwritten
                    if k == 1:
                        nc.vector.tensor_scalar_mul(
                            out=dst, in0=sp[:], scalar1=m1_s[:, t : t + 1])
                    else:
                        st1 = stp.tile([128, 128], f32, tag="st1")
                        nc.vector.tensor_scalar_mul(
                            out=st1[:], in0=sp[:], scalar1=m2_s[:, t : t + 1])
                        nc.vector.tensor_tensor(
                            out=dst, in0=st1[:], in1=prv,
                            op=mybir.AluOpType.subtract)
                    if k < K - 1:
                        xb = stp.tile([128, D], bft, tag="xb")
                        nc.scalar.activation(out=xb[:], in_=dst,
                                             func=mybir.ActivationFunctionType.Copy)
                        nc.sync.dma_start(
                            out=agin[k][128 * t : 128 * (t + 1), :], in_=xb[:])
                    z_project(k, dst, t)
                if k < K - 1:
                    nc.gpsimd.collective_compute(
                        "AllGather", mybir.AluOpType.bypass,
                        replica_groups=[list(range(NCORES))],
                        ins=[agin[k][:, :]],
                        outs=[tables[k][1 : TOT_ROWS - 1, :]],
                    )

            # ---- final: relu, fc, log_softmax ----------------------------
            for t in range(TILES):
                zsl = z_s[:, 128 * t : 128 * (t + 1)]
                hT = finp.tile([50, 128], f32, tag="hT")
                nc.scalar.activation(out=hT[:], in_=zsl,
                                     func=mybir.ActivationFunctionType.Relu,
                                     bias=cb_s[:, 0:1])
                lgp = ps_z.tile([10, 128], f32, space="PSUM", tag="zp")
                nc.tensor.matmul(out=lgp[:], lhsT=fw_s[:], rhs=hT[:],
                                 start=True, stop=True)
                lgs = finp.tile([10, 128], f32, tag="lgs")
                nc.vector.tensor_copy(out=lgs[:], in_=lgp[:])
                ltp = ps_t.tile([128, 10], f32, space="PSUM", tag="tp")
                nc.tensor.transpose(out=ltp[:], in_=lgs[:],
                                    identity=ident[0:10, 0:10])
                L = finp.tile([128, 10], f32, tag="L")
                nc.vector.tensor_tensor(out=L[:], in0=ltp[:], in1=fb_s[:],
                                        op=mybir.AluOpType.add)
                m = finp.tile([128, 1], f32, tag="m")
                nc.vector.tensor_reduce(out=m[:], in_=L[:],
                                        axis=mybir.AxisListType.X,
                                        op=mybir.AluOpType.max)
                negm = finp.tile([128, 1], f32, tag="negm")
                nc.vector.tensor_scalar_mul(out=negm[:], in0=m[:], scalar1=-1.0)
                Ex = finp.tile([128, 10], f32, tag="Ex")
                ssum = finp.tile([128, 1], f32, tag="ssum")
                nc.scalar.activation(out=Ex[:], in_=L[:],
                                     func=mybir.ActivationFunctionType.Exp,
                                     bias=negm[:, 0:1], accum_out=ssum[:])
                lns = finp.tile([128, 1], f32, tag="lns")
                nc.scalar.activation(out=lns[:], in_=ssum[:],
                                     func=mybir.ActivationFunctionType.Ln)
                O = finp.tile([128, 10], f32, tag="O")
                nc.vector.tensor_scalar(out=O[:], in0=L[:],
                                        scalar1=m[:, 0:1], scalar2=lns[:, 0:1],
                                        op0=mybir.AluOpType.subtract,
                                        op1=mybir.AluOpType.subtract)
                nc.sync.dma_start(out=out_d[128 * t : 128 * (t + 1), :], in_=O[:])
    nc.finalize()
    return nc


_CACHED = {}


def kernel(x, edge_index, cheb_w, cheb_b, fc_w, fc_b):
    x = np.ascontiguousarray(np.asarray(x, dtype=np.float32))
    cheb_w = np.asarray(cheb_w, dtype=np.float32)
    cheb_b = np.asarray(cheb_b, dtype=np.float32)
    fc_w = np.asarray(fc_w, dtype=np.float32)
    fc_b = np.asarray(fc_b, dtype=np.float32)

    meta = host_prep(x, edge_index)
    nc = build_nc(meta, cheb_w, cheb_b, fc_w, fc_b)

    # per-core inputs
    wcheb = np.ascontiguousarray(
        cheb_w.transpose(1, 0, 2).reshape(D, K * 50)).astype(np.float32)
    pats_flat = meta["pats"].reshape(-1, 128)
    in_maps = []
    for c in range(NCORES):
        in_maps.append({
            "xp": meta["xp"][c],
            "idx_lo": meta["idx_lo_w"][c],
            "idx_hi": meta["idx_hi_w"][c],
            "pats": pats_flat,
            "dinv_t": meta["dinv_t"][c],
            "m1di2_t": meta["m1di2_t"][c],
            "m2di2_t": meta["m2di2_t"][c],
            "sdeg_t": meta["sdeg_t"][c],
            "wcheb": wcheb,
            "cbias": cheb_b.reshape(50, 1),
            "fcw": fc_w,
            "fcb_rep": np.tile(fc_b.reshape(1, 10), (128, 1)).astype(np.float32),
            "ident": np.eye(128, dtype=np.float32),
        })

    from concourse.bass_utils import run_bass_kernel_spmd
    res = run_bass_kernel_spmd(nc, in_maps, core_ids=list(range(NCORES)))

    out = np.empty((N, 10), dtype=np.float32)
    for c in range(NCORES):
        out[meta["perms"][c]] = res.results[c]["out"][:NPC]
    return out



# revision 7
# speedup vs baseline: 2.3229x; 2.3229x over previous
"""ChebConv(K=5) + Linear + log_softmax GNN kernel for 8 Trainium2 NeuronCores.

Strategy (graph/data parallel, nodes sharded by destination):
 - Clenshaw recurrence: out = sum_k T_k(P) x W_k is evaluated backward as
   b_k = z_k + 2 P b_{k+1} - b_{k+2} with z_k = x @ W_k precomputed on device,
   so every propagation P b = -dinv * gathersum(dinv * b) runs in the 50-dim
   hidden space (4 propagations total, like the forward recurrence).
 - Per hop, each core gathers its edges' source rows from a replicated bf16
   table in HBM via ASYNC gpsimd dma_gather (prepare_only + trigger_dma):
   GpSimd only generates descriptors while the 16 SDMA rings stream data,
   overlapped with TensorE consumption.
 - Interleaved slot layout: a destination tile's edges are stored column-major
   (chunk j holds the j-th edge of all 128 destinations), so the per-chunk
   segment-sum reduces to a PSUM accumulation with a single stationary
   identity lhsT - no pattern pool, ~3% padding (per-tile max degree over the
   8 cores, destinations sorted by degree).
 - int16 gather indices force a lo/hi table split (sources on cores 0-4 vs
   5-7). lo and hi use independently degree-sorted destination orders; hi
   partial sums are staged to DRAM and aligned back to the canonical (lo)
   order with a small permutation gather each hop.
 - Hops are joined by an 8-core AllGather of each core's updated table slice.
"""
import numpy as np
import ml_dtypes

bf16 = ml_dtypes.bfloat16

# ---------------- problem constants (hardcoded per contract) ---------------
N = 50000
E = 1_600_000
D = 128
DH = 50
K = 5
NCORES = 8
NPC = N // NCORES            # 6250
TILES = (NPC + 127) // 128   # 49
S = TILES * 128              # 6272
LO_CORES = 5
LO_SPLIT_NODE = LO_CORES * NPC       # 31250
HI_BASE_ROW = 1 + LO_CORES * S       # 31361
TOT_ROWS = 2 + NCORES * S            # 50178
HI_PAD_IDX = TOT_ROWS - 1 - HI_BASE_ROW  # 18816 -> trailing zero row
G_CH = 32                    # gather chunks per dma_gather call
PG = 8                       # tiles per permutation-gather call
NHOPS = K - 1


def _wrap_idx(a):
    """Flat int16 index array -> dma_gather layout [128, n/16]."""
    t = a.reshape(-1, 16).T
    return np.ascontiguousarray(np.tile(t, (8, 1)))


def host_prep(x, edge_index):
    row = np.ascontiguousarray(edge_index[0]).astype(np.int64)
    col = np.ascontiguousarray(edge_index[1]).astype(np.int64)
    deg = np.bincount(row, minlength=N)
    assert (deg > 0).all(), "kernel assumes no isolated (deg-0) nodes"
    degf = deg.astype(np.float32)
    dinv = (1.0 / np.sqrt(degf)).astype(np.float32)

    is_lo = col < LO_SPLIT_NODE
    lodeg = np.bincount(row[is_lo], minlength=N)
    hideg = deg - lodeg

    # decoupled per-core orderings: canonical = sorted by lo-degree desc
    perms_lo = np.empty((NCORES, NPC), dtype=np.int64)
    perms_hi = np.empty((NCORES, NPC), dtype=np.int64)
    for c in range(NCORES):
        ids = np.arange(c * NPC, (c + 1) * NPC)
        perms_lo[c] = ids[np.argsort(-lodeg[ids], kind="stable")]
        perms_hi[c] = ids[np.argsort(-hideg[ids], kind="stable")]
    rank_lo = np.empty(N, dtype=np.int64)
    rank_hi = np.empty(N, dtype=np.int64)
    rank_lo[perms_lo.reshape(-1)] = np.tile(np.arange(NPC), NCORES)
    rank_hi[perms_hi.reshape(-1)] = np.tile(np.arange(NPC), NCORES)
    pid = 1 + (np.arange(N) // NPC) * S + rank_lo  # table row of each node

    # per-tile chunk counts: max degree in tile, maxed over cores
    lodeg_sorted = lodeg[perms_lo]  # [NCORES, NPC] descending per core
    hideg_sorted = hideg[perms_hi]
    CLO = np.array([int(lodeg_sorted[:, 128 * t].max()) for t in range(TILES)])
    CHI = np.array([int(hideg_sorted[:, 128 * t].max()) for t in range(TILES)])
    assert (CLO > 0).all() and (CHI > 0).all()
    lo_off = np.zeros(TILES + 1, dtype=np.int64)
    hi_off = np.zeros(TILES + 1, dtype=np.int64)
    np.cumsum(CLO, out=lo_off[1:])
    np.cumsum(CHI, out=hi_off[1:])
    nch_lo, nch_hi = int(lo_off[-1]), int(hi_off[-1])
    n_lo, n_hi = 128 * nch_lo, 128 * nch_hi

    # edges sorted by dst; occurrence index within (dst, lo/hi)
    order_e = np.argsort(row, kind="stable")
    row_s, col_s = row[order_e], col[order_e]
    is_lo_s = is_lo[order_e]

    def occ_index(dst_sub, count_sub):
        st = np.zeros(N + 1, dtype=np.int64)
        np.cumsum(count_sub, out=st[1:])
        return np.arange(dst_sub.shape[0], dtype=np.int64) - st[dst_sub]

    dst_lo, src_lo = row_s[is_lo_s], col_s[is_lo_s]
    dst_hi, src_hi = row_s[~is_lo_s], col_s[~is_lo_s]
    j_lo = occ_index(dst_lo, np.bincount(dst_lo, minlength=N))
    j_hi = occ_index(dst_hi, np.bincount(dst_hi, minlength=N))

    # interleaved slot layout: slot(t, j, r) at (off[t]+j)*128 + r
    idx_lo = np.zeros((NCORES, n_lo), dtype=np.int16)             # pad -> row 0
    idx_hi = np.full((NCORES, n_hi), HI_PAD_IDX, dtype=np.int16)  # pad -> zero row
    core_lo, core_hi = dst_lo // NPC, dst_hi // NPC
    r_lo, r_hi = rank_lo[dst_lo], rank_hi[dst_hi]
    slot_lo = (lo_off[r_lo // 128] + j_lo) * 128 + r_lo % 128
    slot_hi = (hi_off[r_hi // 128] + j_hi) * 128 + r_hi % 128
    v_lo = pid[src_lo]
    v_hi = pid[src_hi] - HI_BASE_ROW
    assert v_lo.max() <= 32767 and v_lo.min() >= 1
    assert v_hi.max() <= 32767 and v_hi.min() >= 0
    assert (j_lo < CLO[r_lo // 128]).all() and (j_hi < CHI[r_hi // 128]).all()
    idx_lo[core_lo, slot_lo] = v_lo.astype(np.int16)
    idx_hi[core_hi, slot_hi] = v_hi.astype(np.int16)

    # hi->lo permutation: canonical (lo-order) rank g takes hi-sum row
    # rank_hi[node at lo-rank g]; pad rows map to themselves
    idx_pm = np.tile(np.arange(S, dtype=np.int16), (NCORES, 1))
    for c in range(NCORES):
        idx_pm[c, :NPC] = rank_hi[perms_lo[c]].astype(np.int16)

    idx_lo_w = np.stack([_wrap_idx(idx_lo[c]) for c in range(NCORES)])
    idx_hi_w = np.stack([_wrap_idx(idx_hi[c]) for c in range(NCORES)])
    idx_pm_w = np.stack([_wrap_idx(idx_pm[c]) for c in range(NCORES)])

    # per-row constants in [128, TILES] layout (canonical order)
    def rowconst(vals_percore):  # [NCORES, S] f32 -> [NCORES, 128, TILES]
        return np.ascontiguousarray(
            vals_percore.reshape(NCORES, TILES, 128).transpose(0, 2, 1))

    dinv_p = np.zeros((NCORES, S), dtype=np.float32)
    for c in range(NCORES):
        dinv_p[c, :NPC] = dinv[perms_lo[c]]

    xp = np.zeros((NCORES, S, D), dtype=np.float32)
    for c in range(NCORES):
        xp[c, :NPC] = x[perms_lo[c]]

    return dict(
        perms=perms_lo, CLO=CLO, CHI=CHI,
        lo_off=lo_off, hi_off=hi_off,
        nch_lo=nch_lo, nch_hi=nch_hi, n_lo=n_lo, n_hi=n_hi,
        idx_lo_w=idx_lo_w, idx_hi_w=idx_hi_w, idx_pm_w=idx_pm_w,
        xp=xp,
        dinv_t=rowconst(dinv_p),
        md2_t=rowconst(-2.0 * dinv_p),
        md1_t=rowconst(-dinv_p),
    )


def _chunk_units(C, off):
    """[(tile, chunk_in_tile, is_first, is_last)] in flat chunk order."""
    units = []
    for t in range(TILES):
        for j in range(int(C[t])):
            units.append((t, j, j == 0, j == int(C[t]) - 1))
    return units


def build_nc(meta):
    from concourse import bacc, mybir
    import concourse.tile as tile

    f32, bft, i16 = mybir.dt.float32, mybir.dt.bfloat16, mybir.dt.int16
    CLO, CHI = meta["CLO"], meta["CHI"]
    n_lo, n_hi = meta["n_lo"], meta["n_hi"]
    nch_lo, nch_hi = meta["nch_lo"], meta["nch_hi"]

    nc = bacc.Bacc(target_bir_lowering=False, num_swdge_queues=4)

    # ---- I/O --------------------------------------------------------------
    xp_d = nc.declare_dram_parameter("xp", [S, D], f32, isOutput=False)
    il_d = nc.declare_dram_parameter("idx_lo", [128, n_lo // 16], i16, isOutput=False)
    ih_d = nc.declare_dram_parameter("idx_hi", [128, n_hi // 16], i16, isOutput=False)
    ip_d = nc.declare_dram_parameter("idx_pm", [128, S // 16], i16, isOutput=False)
    dinv_d = nc.declare_dram_parameter("dinv_t", [128, TILES], f32, isOutput=False)
    md2_d = nc.declare_dram_parameter("md2_t", [128, TILES], f32, isOutput=False)
    md1_d = nc.declare_dram_parameter("md1_t", [128, TILES], f32, isOutput=False)
    wch_d = nc.declare_dram_parameter("wch", [D, K * DH], f32, isOutput=False)
    cb_d = nc.declare_dram_parameter("cbias", [DH, 1], f32, isOutput=False)
    fw_d = nc.declare_dram_parameter("fcw", [DH, 10], f32, isOutput=False)
    fb_d = nc.declare_dram_parameter("fcb_rep", [128, 10], f32, isOutput=False)
    id_d = nc.declare_dram_parameter("ident", [128, 128], f32, isOutput=False)
    idb_d = nc.declare_dram_parameter("identb", [128, 128], bft, isOutput=False)
    out_d = nc.declare_dram_parameter("out", [S, 10], f32, isOutput=True)

    # ---- internal DRAM ----------------------------------------------------
    agin = [nc.dram_tensor(f"agin{j}", [S, D], bft) for j in range(NHOPS)]
    tables = [
        nc.dram_tensor(f"table{j}", [TOT_ROWS, D], bft, addr_space="Shared")
        for j in range(NHOPS)
    ]
    hs_d = [nc.dram_tensor(f"histage{i}", [S, 64], f32) for i in range(2)]

    lo_units = _chunk_units(CLO, meta["lo_off"])
    hi_units = _chunk_units(CHI, meta["hi_off"])

    with tile.TileContext(nc) as tc:
        with tc.tile_pool(name="cst", bufs=1) as cst, \
             tc.tile_pool(name="xt", bufs=3) as xtp, \
             tc.tile_pool(name="xtt", bufs=2) as xttp, \
             tc.tile_pool(name="glo", bufs=3) as glop, \
             tc.tile_pool(name="ghi", bufs=3) as ghip, \
             tc.tile_pool(name="hpm", bufs=2) as hpmp, \
             tc.tile_pool(name="st", bufs=6) as stp, \
             tc.tile_pool(name="ub", bufs=3) as ubp, \
             tc.tile_pool(name="fin", bufs=2) as finp, \
             tc.tile_pool(name="ps_t", bufs=2, space="PSUM") as ps_t, \
             tc.tile_pool(name="ps_z", bufs=2, space="PSUM") as ps_z, \
             tc.tile_pool(name="ps_lo", bufs=2, space="PSUM") as ps_lo, \
             tc.tile_pool(name="ps_hi", bufs=2, space="PSUM") as ps_hi:

            # ---- resident constants --------------------------------------
            idx_lo_s = cst.tile([128, n_lo // 16], i16)
            idx_hi_s = cst.tile([128, n_hi // 16], i16)
            idx_pm_s = cst.tile([128, S // 16], i16)
            nc.sync.dma_start(out=idx_lo_s[:], in_=il_d[:, :])
            nc.sync.dma_start(out=idx_hi_s[:], in_=ih_d[:, :])
            nc.sync.dma_start(out=idx_pm_s[:], in_=ip_d[:, :])
            ident = cst.tile([128, 128], f32)
            nc.sync.dma_start(out=ident[:], in_=id_d[:, :])
            identb = cst.tile([128, 128], bft)
            nc.sync.dma_start(out=identb[:], in_=idb_d[:, :])
            dinv_s = cst.tile([128, TILES], f32)
            nc.sync.dma_start(out=dinv_s[:], in_=dinv_d[:, :])
            md2_s = cst.tile([128, TILES], f32)
            nc.sync.dma_start(out=md2_s[:], in_=md2_d[:, :])
            md1_s = cst.tile([128, TILES], f32)
            nc.sync.dma_start(out=md1_s[:], in_=md1_d[:, :])
            wch_s = cst.tile([128, K * DH], f32)
            nc.sync.dma_start(out=wch_s[:], in_=wch_d[:, :])
            cb_s = cst.tile([DH, 1], f32)
            nc.sync.dma_start(out=cb_s[:], in_=cb_d[:, :])
            fw_s = cst.tile([DH, 10], f32)
            nc.sync.dma_start(out=fw_s[:], in_=fw_d[:, :])
            fb_s = cst.tile([128, 10], f32)
            nc.sync.dma_start(out=fb_s[:], in_=fb_d[:, :])

            z_s = cst.tile([128, K, TILES, DH], f32)    # z_k = x @ W_k
            bA = cst.tile([128, TILES, DH], f32)        # b_3 then b_1
            bB = cst.tile([128, TILES, DH], f32)        # b_2
            slo = cst.tile([128, TILES, DH], f32)       # lo partial sums

            # zero rows of each table
            zrow = cst.tile([1, D], bft)
            nc.vector.memset(zrow[:], 0.0)
            for t in tables:
                nc.sync.dma_start(out=t[0:1, :], in_=zrow[:])
                nc.sync.dma_start(out=t[TOT_ROWS - 1 : TOT_ROWS, :], in_=zrow[:])

            # ---- prologue: z_k = x @ W_k; table0 from u(b_4 = z_4) -------
            for t in range(TILES):
                xt = xtp.tile([128, D], f32)
                nc.sync.dma_start(out=xt[:], in_=xp_d[128 * t : 128 * (t + 1), :])
                tp = ps_t.tile([128, 128], f32, space="PSUM")
                nc.tensor.transpose(out=tp[:], in_=xt[:], identity=ident[:])
                xT = xttp.tile([128, 128], f32)
                nc.vector.tensor_copy(out=xT[:], in_=tp[:])
                for k in range(K):
                    zp = ps_z.tile([128, DH], f32, space="PSUM")
                    nc.tensor.matmul(out=zp[:], lhsT=xT[:],
                                     rhs=wch_s[:, DH * k : DH * (k + 1)],
                                     start=True, stop=True)
                    nc.vector.tensor_copy(out=z_s[:, k, t, :], in_=zp[:])
                u1 = stp.tile([128, DH], f32, tag="u1")
                nc.vector.tensor_scalar_mul(
                    out=u1[:], in0=z_s[:, K - 1, t, :],
                    scalar1=dinv_s[:, t : t + 1])
                ut = ubp.tile([128, D], bft)
                nc.scalar.activation(out=ut[:, 0:DH], in_=u1[:],
                                     func=mybir.ActivationFunctionType.Copy)
                nc.vector.memset(ut[:, DH:D], 0.0)
                nc.sync.dma_start(out=agin[0][128 * t : 128 * (t + 1), :], in_=ut[:])
            nc.gpsimd.collective_compute(
                "AllGather", mybir.AluOpType.bypass,
                replica_groups=[list(range(NCORES))],
                ins=[agin[0][:, :]], outs=[tables[0][1 : TOT_ROWS - 1, :]],
            )

            # ---- hops ----------------------------------------------------
            qn = 0  # global Pool-DMA counter: queue k%4 matches lane k%8
            for j in range(NHOPS):
                tbl = tables[j]
                tbl_hi = tbl[HI_BASE_ROW:TOT_ROWS, :]
                zk = NHOPS - 1 - j  # z index joined this hop (3,2,1,0)
                hs = hs_d[j % 2]

                # interleave lo/hi gather groups
                lo_groups = [(g, min(G_CH, nch_lo - g))
                             for g in range(0, nch_lo, G_CH)]
                hi_groups = [(g, min(G_CH, nch_hi - g))
                             for g in range(0, nch_hi, G_CH)]
                sched = []
                li = hi = 0
                while li < len(lo_groups) or hi < len(hi_groups):
                    if li < len(lo_groups):
                        sched.append(("lo", lo_groups[li])); li += 1
                    if hi < len(hi_groups):
                        sched.append(("hi", hi_groups[hi])); hi += 1

                psum_open = {}
                for kind, (gstart, gn) in sched:
                    if kind == "lo":
                        gt = glop.tile([128, G_CH, 128], bft)
                        nc.gpsimd.dma_gather(
                            out_ap=gt[:, :gn, :], in_ap=tbl[:, :],
                            idxs_ap=idx_lo_s[:, 8 * gstart : 8 * (gstart + gn)],
                            num_idxs=gn * 128, num_idxs_reg=gn * 128,
                            elem_size=D, queue_num=qn % 4,
                            single_packet=False,
                        )
                        qn += 1
                        units, pool = lo_units, ps_lo
                    else:
                        gt = ghip.tile([128, G_CH, 128], bft)
                        nc.gpsimd.dma_gather(
                            out_ap=gt[:, :gn, :], in_ap=tbl_hi,
                            idxs_ap=idx_hi_s[:, 8 * gstart : 8 * (gstart + gn)],
                            num_idxs=gn * 128, num_idxs_reg=gn * 128,
                            elem_size=D, queue_num=qn % 4,
                            single_packet=False,
                        )
                        qn += 1
                        units, pool = hi_units, ps_hi

                    for ci in range(gstart, gstart + gn):
                        t, cj, first, last = units[ci]
                        key = (kind, t)
                        if first:
                            psum_open[key] = pool.tile([128, DH], f32,
                                                       space="PSUM",
                                                       name=f"acc_{kind}",
                                                       tag=f"acc_{kind}")
                        nc.tensor.matmul(
                            out=psum_open[key][:], lhsT=identb[:],
                            rhs=gt[:, ci - gstart, 0:DH],
                            start=first, stop=last, skip_group_check=True,
                        )
                        if last:
                            pl = psum_open.pop(key)
                            if kind == "lo":
                                nc.vector.tensor_copy(out=slo[:, t, :], in_=pl[:])
                            else:
                                sh = stp.tile([128, 64], f32, tag="sh")
                                nc.vector.tensor_copy(out=sh[:, 0:DH], in_=pl[:])
                                nc.vector.memset(sh[:, DH:64], 0.0)
                                nc.sync.dma_start(
                                    out=hs[128 * t : 128 * (t + 1), :],
                                    in_=sh[:])
                assert not psum_open

                # permutation gathers: hi sums (hi order) -> canonical order
                for pt in range(0, TILES, PG):
                    pn = min(PG, TILES - pt)
                    ht = hpmp.tile([128, PG, 64], f32)
                    nc.gpsimd.dma_gather(
                        out_ap=ht[:, :pn, :], in_ap=hs[:, :],
                        idxs_ap=idx_pm_s[:, 8 * pt : 8 * (pt + pn)],
                        num_idxs=pn * 128, num_idxs_reg=pn * 128,
                        elem_size=64, queue_num=qn % 4,
                        single_packet=False,
                    )
                    qn += 1

                    for t in range(pt, pt + pn):
                        H = ht[:, t - pt, 0:DH]
                        stt = stp.tile([128, DH], f32, tag="stt")
                        nc.vector.tensor_tensor(
                            out=stt[:], in0=slo[:, t, :], in1=H,
                            op=mybir.AluOpType.add)
                        if j < NHOPS - 1:
                            dst = (bA if j % 2 == 0 else bB)[:, t, :]
                            t1 = stp.tile([128, DH], f32, tag="t1")
                            nc.vector.tensor_scalar_mul(
                                out=t1[:], in0=stt[:],
                                scalar1=md2_s[:, t : t + 1])
                            if j == 0:
                                nc.vector.tensor_tensor(
                                    out=dst, in0=t1[:], in1=z_s[:, zk, t, :],
                                    op=mybir.AluOpType.add)
                            else:
                                prev = (z_s[:, K - 1, t, :] if j == 1
                                        else bA[:, t, :])
                                t2 = stp.tile([128, DH], f32, tag="t2")
                                nc.vector.tensor_tensor(
                                    out=t2[:], in0=t1[:], in1=z_s[:, zk, t, :],
                                    op=mybir.AluOpType.add)
                                nc.vector.tensor_tensor(
                                    out=dst, in0=t2[:], in1=prev,
                                    op=mybir.AluOpType.subtract)
                            u1 = stp.tile([128, DH], f32, tag="u1")
                            nc.vector.tensor_scalar_mul(
                                out=u1[:], in0=dst,
                                scalar1=dinv_s[:, t : t + 1])
                            ut = ubp.tile([128, D], bft)
                            nc.scalar.activation(
                                out=ut[:, 0:DH], in_=u1[:],
                                func=mybir.ActivationFunctionType.Copy)
                            nc.vector.memset(ut[:, DH:D], 0.0)
                            nc.sync.dma_start(
                                out=agin[j + 1][128 * t : 128 * (t + 1), :],
                                in_=ut[:])
                        else:
                            # out_pre = z_0 - dinv*s - b_2; then head
                            t1 = stp.tile([128, DH], f32, tag="t1")
                            nc.vector.tensor_scalar_mul(
                                out=t1[:], in0=stt[:],
                                scalar1=md1_s[:, t : t + 1])
                            t2 = stp.tile([128, DH], f32, tag="t2")
                            nc.vector.tensor_tensor(
                                out=t2[:], in0=t1[:], in1=z_s[:, 0, t, :],
                                op=mybir.AluOpType.add)
                            op_ = stp.tile([128, DH], f32, tag="op")
                            nc.vector.tensor_tensor(
                                out=op_[:], in0=t2[:], in1=bB[:, t, :],
                                op=mybir.AluOpType.subtract)
                            tpz = ps_t.tile([DH, 128], f32, space="PSUM",
                                            name="tp", tag="tp")
                            nc.tensor.transpose(out=tpz[:], in_=op_[:],
                                                identity=ident[:])
                            hT = finp.tile([DH, 128], f32, tag="hT")
                            nc.scalar.activation(
                                out=hT[:], in_=tpz[:],
                                func=mybir.ActivationFunctionType.Relu,
                                bias=cb_s[:, 0:1])
                            lgp = ps_z.tile([10, 128], f32, space="PSUM",
                                            name="zp", tag="zp")
                            nc.tensor.matmul(out=lgp[:], lhsT=fw_s[:],
                                             rhs=hT[:], start=True, stop=True)
                            lgs = finp.tile([10, 128], f32, tag="lgs")
                            nc.vector.tensor_copy(out=lgs[:], in_=lgp[:])
                            ltp = ps_t.tile([128, 10], f32, space="PSUM",
                                            name="tp", tag="tp")
                            nc.tensor.transpose(out=ltp[:], in_=lgs[:],
                                                identity=ident[0:10, 0:10])
                            L = finp.tile([128, 10], f32, tag="L")
                            nc.vector.tensor_tensor(out=L[:], in0=ltp[:],
                                                    in1=fb_s[:],
                                                    op=mybir.AluOpType.add)
                            m = finp.tile([128, 1], f32, tag="m")
                            nc.vector.tensor_reduce(out=m[:], in_=L[:],
                                                    axis=mybir.AxisListType.X,
                                                    op=mybir.AluOpType.max)
                            negm = finp.tile([128, 1], f32, tag="negm")
                            nc.vector.tensor_scalar_mul(out=negm[:], in0=m[:],
                                                        scalar1=-1.0)
                            Ex = finp.tile([128, 10], f32, tag="Ex")
                            ssum = finp.tile([128, 1], f32, tag="ssum")
                            nc.scalar.activation(
                                out=Ex[:], in_=L[:],
                                func=mybir.ActivationFunctionType.Exp,
                                bias=negm[:, 0:1], accum_out=ssum[:])
                            lns = finp.tile([128, 1], f32, tag="lns")
                            nc.scalar.activation(
                                out=lns[:], in_=ssum[:],
                                func=mybir.ActivationFunctionType.Ln)
                            O = finp.tile([128, 10], f32, tag="O")
                            nc.vector.tensor_scalar(
                                out=O[:], in0=L[:],
                                scalar1=m[:, 0:1], scalar2=lns[:, 0:1],
                                op0=mybir.AluOpType.subtract,
                                op1=mybir.AluOpType.subtract)
                            nc.sync.dma_start(
                                out=out_d[128 * t : 128 * (t + 1), :],
                                in_=O[:])
                if j < NHOPS - 1:
                    nc.gpsimd.collective_compute(
                        "AllGather", mybir.AluOpType.bypass,
                        replica_groups=[list(range(NCORES))],
                        ins=[agin[j + 1][:, :]],
                        outs=[tables[j + 1][1 : TOT_ROWS - 1, :]],
                    )
    nc.finalize()
    return nc


def make_in_maps(meta, cheb_w, cheb_b, fc_w, fc_b):
    cheb_w = np.asarray(cheb_w, dtype=np.float32)
    wch = np.ascontiguousarray(
        cheb_w.transpose(1, 0, 2).reshape(D, K * DH)).astype(np.float32)
    shared = {
        "wch": wch,
        "cbias": np.asarray(cheb_b, dtype=np.float32).reshape(DH, 1),
        "fcw": np.asarray(fc_w, dtype=np.float32),
        "fcb_rep": np.tile(np.asarray(fc_b, dtype=np.float32).reshape(1, 10),
                           (128, 1)),
        "ident": np.eye(128, dtype=np.float32),
        "identb": np.eye(128, dtype=bf16),
    }
    in_maps = []
    for c in range(NCORES):
        m = dict(shared)
        m.update({
            "xp": meta["xp"][c],
            "idx_lo": meta["idx_lo_w"][c],
            "idx_hi": meta["idx_hi_w"][c],
            "idx_pm": meta["idx_pm_w"][c],
            "dinv_t": meta["dinv_t"][c],
            "md2_t": meta["md2_t"][c],
            "md1_t": meta["md1_t"][c],
        })
        in_maps.append(m)
    return in_maps


def kernel(x, edge_index, cheb_w, cheb_b, fc_w, fc_b):
    x = np.ascontiguousarray(np.asarray(x, dtype=np.float32))
    meta = host_prep(x, np.asarray(edge_index))
    nc = build_nc(meta)
    in_maps = make_in_maps(meta, cheb_w, cheb_b, fc_w, fc_b)

    from concourse.bass_utils import run_bass_kernel_spmd
    res = run_bass_kernel_spmd(nc, in_maps, core_ids=list(range(NCORES)))

    out = np.empty((N, 10), dtype=np.float32)
    for c in range(NCORES):
        out[meta["perms"][c]] = res.results[c]["out"][:NPC]
    return out
